# revision 1
# baseline (speedup 1.0000x reference)
"""Trainium2 Bass kernel for nn_Differentiable_Global_Geometry_PointCloud.

Full pipeline on 8 NeuronCores, data-parallel over the 4096 points (512/core):
  - negD distance matrix via PE (5-row contraction)
  - exact top-50 KNN via per-chunk max8/match_replace + fp32-packed-key extraction
  - 3x3 eigh via faithful branchless fp32 port of LAPACK 3.12 ssyevd
    (ssytd2 + ssteqr QL-with-reversal + slaev2 + sort + sormtr) -> exact sign
    reproduction of the scipy/jax CPU reference
  - neighbor coord/normal fetch via dma_gather on 256B padded records
  - per-point tangent projections, Voronoi area via per-row interval
    intersection (convexity of the cell), Weingarten curvature
Host only shards inputs / sums the 8 partial outputs.
"""
import os
import sys

import numpy as np

for _p in ("/opt/trn_rl_repo", "/opt/trn_rl_repo/concourse",
           os.path.expanduser("~/.axon_site/_ro/trn_rl_repo"),
           os.path.expanduser("~/.axon_site/_ro/trn_rl_repo/concourse")):
    if os.path.isdir(_p) and _p not in sys.path:
        sys.path.insert(0, _p)

import concourse.bass as bass
import concourse.bacc as bacc
import concourse.mybir as mybir
from concourse.tile import TileContext
from concourse import library_config

f32 = np.float32
FP = mybir.dt.float32
U32 = mybir.dt.uint32
I16 = mybir.dt.int16
OP = mybir.AluOpType
AF = mybir.ActivationFunctionType
AX = mybir.AxisListType

N = 4096          # total points
NCORE = 8
NPC = N // NCORE  # 512 points per core
NT = NPC // 128   # 4 q-tiles per core
KSL = 56          # neighbor slots kept (>= 50)
CH = 512          # selection chunk width
NCH = N // CH
TOPT = 16         # per-chunk extraction depth
W = 64            # voronoi grid width
EPS2 = f32(2.0 ** -48)
SAFMIN = f32(2.0 ** -126)

YGRID_BITS = [3212836864, 3212304254, 3211771644, 3211239034, 3210706424, 3210173814, 3209641204, 3209108594, 3208575984, 3208043374, 3207510764, 3206978154, 3206445544, 3205912933, 3205380324, 3204847713, 3204181951, 3203116729, 3202051511, 3200986289, 3199921071, 3198855849, 3197790631, 3196725409, 3195260729, 3193130289, 3190999849, 3188869409, 3185806897, 3181546017, 3175287841, 3162638337, 1015154752, 1027804224, 1034062384, 1038323265, 1041385768, 1043516209, 1045646649, 1047777089, 1049241764, 1050306985, 1051372205, 1052437425, 1053502645, 1054567865, 1055633085, 1056698305, 1057364068, 1057896678, 1058429288, 1058961898, 1059494508, 1060027118, 1060559728, 1061092338, 1061624948, 1062157558, 1062690168, 1063222778, 1063755388, 1064287998, 1064820608, 1065353216]
YGRID = np.array(YGRID_BITS, np.uint32).view(np.float32)

_nc_cache = {}


class Emit:
    """Helper wrapping plane-algebra emission on [128, w] fp32 tiles."""

    def __init__(self, nc, tc, pool):
        self.nc = nc
        self.tc = tc
        self.pool = pool
        self.n = 0
        self.consts = {}

    def fresh(self, w=4, dtype=FP):
        self.n += 1
        if w <= 12:
            tag = f"rn{self.n % 320}"
        else:
            tag = f"rw{self.n % 64}"
        return self.pool.tile([128, w], dtype, tag=tag, name=f"f{self.n}")

    def named(self, w=4, dtype=FP, name="nm"):
        return self.pool.tile([128, w], dtype, tag=name, name=name)

    def cplane(self, val, w=4):
        key = (float(val), w)
        if key not in self.consts:
            nm = f"c{len(self.consts)}_{w}"
            t = self.pool.tile([128, w], FP, tag=nm, name=nm)
            self.nc.vector.memset(t[:], float(val))
            self.consts[key] = t
        return self.consts[key]

    def _ap(self, x):
        return x if isinstance(x, bass.AP) else x[:]

    def tt(self, a, b, op, out=None, w=None):
        a = self._ap(a); b = self._ap(b)
        if op == OP.divide:
            rb = self.fresh(b.free_size())
            self.nc.vector.reciprocal(rb[:], b)
            b = rb[:]
            op = OP.mult
        if out is None:
            out = self.fresh(w or a.shape[-1] if len(a.shape) == 2 else a.free_size())
        self.nc.vector.tensor_tensor(out=self._ap(out), in0=a, in1=b, op=op)
        return out

    def ts(self, a, s, op, s2=None, op2=None, out=None, w=None):
        a = self._ap(a)
        if out is None:
            out = self.fresh(w or a.free_size())
        if op == OP.divide and not isinstance(s, (int, float)):
            sap = self._ap(s)
            rs = self.fresh(sap.free_size())
            self.nc.vector.reciprocal(rs[:], sap)
            s = rs
            op = OP.mult
        s = self._ap(s) if not isinstance(s, (int, float)) else float(s)
        kw = {}
        if s2 is not None:
            kw = dict(scalar2=float(s2) if isinstance(s2, (int, float)) else self._ap(s2), op1=op2)
        else:
            kw = dict(scalar2=None)
        self.nc.vector.tensor_scalar(out=self._ap(out), in0=a, scalar1=s, op0=op, **kw)
        return out

    def stt(self, a, s, b, op0, op1, out=None, w=None):
        a = self._ap(a); b = self._ap(b)
        if out is None:
            out = self.fresh(w or a.free_size())
        s = self._ap(s) if not isinstance(s, (int, float)) else float(s)
        self.nc.vector.scalar_tensor_tensor(out=self._ap(out), in0=a, scalar=s, in1=b, op0=op0, op1=op1)
        return out

    def sel(self, m, a, b, out=None, w=None):
        m = self._ap(m)
        wv = w or m.free_size()
        if out is None:
            out = self.fresh(wv)
        self.n += 1
        mu8 = self.pool.tile([128, wv], mybir.dt.uint8,
                             tag=f"m8{self.n % 48}", name=f"m8_{self.n}")
        self.nc.vector.tensor_copy(mu8[:], m)
        self.nc.vector.select(out=self._ap(out), mask=mu8[:], on_true=self._ap(a), on_false=self._ap(b))
        return out

    def sqrt(self, a, out=None, w=None):
        a = self._ap(a)
        if out is None:
            out = self.fresh(w or a.free_size())
        self.nc.scalar.activation(self._ap(out), a, AF.Sqrt)
        return out

    def recip(self, a, out=None, w=None):
        a = self._ap(a)
        if out is None:
            out = self.fresh(w or a.free_size())
        self.nc.vector.reciprocal(self._ap(out), a)
        return out

    def copy(self, a, out=None, w=None, dtype=FP):
        a = self._ap(a)
        if out is None:
            out = self.fresh(w or a.free_size(), dtype)
        self.nc.vector.tensor_copy(self._ap(out), a)
        return out

    def abs(self, a, out=None, w=None):
        a = self._ap(a)
        if out is None:
            out = self.fresh(w or a.free_size())
        self.nc.scalar.activation(self._ap(out), a, AF.Abs)
        return out

    def reduce(self, a, op, out=None, negate=False):
        a = self._ap(a)
        if out is None:
            out = self.fresh(1)
        self.nc.vector.tensor_reduce(out=self._ap(out), in_=a, axis=AX.X, op=op, negate=negate)
        return out


def slartg_em(E, fv, gv, w=4):
    """LAPACK 3.12 slartg, branchless. fv/gv planes -> (c, s, r)."""
    f1 = E.abs(fv, w=w); g1 = E.abs(gv, w=w)
    ff = E.tt(fv, fv, OP.mult, w=w)
    gg = E.tt(gv, gv, OP.mult, w=w)
    ss_ = E.tt(ff, gg, OP.add, w=w)
    dd = E.sqrt(ss_, w=w)
    dz = E.ts(dd, 0.0, OP.is_equal, w=w)
    dsafe = E.sel(dz, E.cplane(1.0, w), dd, w=w)
    c = E.tt(f1, dsafe, OP.divide, w=w)
    s = E.tt(gv, dsafe, OP.divide, w=w)
    fneg = E.ts(fv, 0.0, OP.is_lt, w=w)
    sn = E.ts(s, -1.0, OP.mult, w=w)
    s = E.sel(fneg, sn, s, w=w)
    ddn = E.ts(dd, -1.0, OP.mult, w=w)
    r = E.sel(fneg, ddn, dd, w=w)
    gz = E.ts(gv, 0.0, OP.is_equal, w=w)
    c = E.sel(gz, E.cplane(1.0, w), c, w=w)
    s = E.sel(gz, E.cplane(0.0, w), s, w=w)
    r = E.sel(gz, fv, r, w=w)
    fz0 = E.ts(fv, 0.0, OP.is_equal, w=w)
    gnz = E.ts(gz, 0.0, OP.is_equal, w=w)
    fz = E.tt(fz0, gnz, OP.mult, w=w)
    sgn_g = E.ts(gv, 0.0, OP.is_ge, s2=2.0, op2=OP.mult, w=w)
    sgn_g = E.ts(sgn_g, -1.0, OP.add, w=w)
    c = E.sel(fz, E.cplane(0.0, w), c, w=w)
    s = E.sel(fz, sgn_g, s, w=w)
    r = E.sel(fz, g1, r, w=w)
    return c, s, r


def slapy2_em(E, xv, yv, w=4):
    xa = E.abs(xv, w=w); ya = E.abs(yv, w=w)
    wv = E.tt(xa, ya, OP.max, w=w)
    zv = E.tt(xa, ya, OP.min, w=w)
    wz = E.ts(wv, 0.0, OP.is_equal, w=w)
    wsafe = E.sel(wz, E.cplane(1.0, w), wv, w=w)
    t = E.tt(zv, wsafe, OP.divide, w=w)
    t2 = E.tt(t, t, OP.mult, w=w)
    t2 = E.ts(t2, 1.0, OP.add, w=w)
    sq = E.sqrt(t2, w=w)
    res = E.tt(wv, sq, OP.mult, w=w)
    zz = E.ts(zv, 0.0, OP.is_equal, w=w)
    return E.sel(zz, wv, res, w=w)


def slaev2_em(E, a, b, c, w=4):
    sm = E.tt(a, c, OP.add, w=w)
    df = E.tt(a, c, OP.subtract, w=w)
    adf = E.abs(df, w=w)
    tb = E.tt(b, b, OP.add, w=w)
    ab = E.abs(tb, w=w)
    aa = E.abs(a, w=w); ac = E.abs(c, w=w)
    agt = E.tt(aa, ac, OP.is_gt, w=w)
    acmx = E.sel(agt, a, c, w=w)
    acmn = E.sel(agt, c, a, w=w)
    adfz = E.ts(adf, 0.0, OP.is_equal, w=w)
    adfs = E.sel(adfz, E.cplane(1.0, w), adf, w=w)
    tgt = E.tt(ab, adfs, OP.divide, w=w)
    tgt2 = E.tt(tgt, tgt, OP.mult, w=w)
    tgt2 = E.ts(tgt2, 1.0, OP.add, w=w)
    rt_gt = E.tt(adf, E.sqrt(tgt2, w=w), OP.mult, w=w)
    abz = E.ts(ab, 0.0, OP.is_equal, w=w)
    abs_ = E.sel(abz, E.cplane(1.0, w), ab, w=w)
    tlt = E.tt(adf, abs_, OP.divide, w=w)
    tlt2 = E.tt(tlt, tlt, OP.mult, w=w)
    tlt2 = E.ts(tlt2, 1.0, OP.add, w=w)
    rt_lt = E.tt(ab, E.sqrt(tlt2, w=w), OP.mult, w=w)
    rt_eq = E.ts(ab, float(np.float32(np.sqrt(np.float32(2.0)))), OP.mult, w=w)
    gtm = E.tt(adf, ab, OP.is_gt, w=w)
    ltm = E.tt(adf, ab, OP.is_lt, w=w)
    rt = E.sel(gtm, rt_gt, E.sel(ltm, rt_lt, rt_eq, w=w), w=w)
    smneg = E.ts(sm, 0.0, OP.is_lt, w=w)
    smpos = E.ts(sm, 0.0, OP.is_gt, w=w)
    smr = E.tt(sm, rt, OP.subtract, w=w)
    rt1n = E.ts(smr, 0.5, OP.mult, w=w)
    sma = E.tt(sm, rt, OP.add, w=w)
    rt1p = E.ts(sma, 0.5, OP.mult, w=w)
    rt1h = E.ts(rt, 0.5, OP.mult, w=w)
    rt1 = E.sel(smneg, rt1n, E.sel(smpos, rt1p, rt1h, w=w), w=w)
    rt1z = E.ts(rt1, 0.0, OP.is_equal, w=w)
    rt1s = E.sel(rt1z, E.cplane(1.0, w), rt1, w=w)
    q1 = E.tt(acmx, rt1s, OP.divide, w=w)
    q1 = E.tt(q1, acmn, OP.mult, w=w)
    q2 = E.tt(b, rt1s, OP.divide, w=w)
    q2 = E.tt(q2, b, OP.mult, w=w)
    rt2g = E.tt(q1, q2, OP.subtract, w=w)
    rt2h = E.ts(rt, -0.5, OP.mult, w=w)
    smnz = E.tt(smneg, smpos, OP.add, w=w)
    rt2 = E.sel(smnz, rt2g, rt2h, w=w)
    sgn1 = E.ts(smneg, -2.0, OP.mult, s2=1.0, op2=OP.add, w=w)   # -1 if neg else +1
    dge = E.ts(df, 0.0, OP.is_ge, w=w)
    cs = E.sel(dge, E.tt(df, rt, OP.add, w=w), E.tt(df, rt, OP.subtract, w=w), w=w)
    sgn2 = E.ts(dge, 2.0, OP.mult, s2=-1.0, op2=OP.add, w=w)
    acs = E.abs(cs, w=w)
    csz = E.ts(cs, 0.0, OP.is_equal, w=w)
    css = E.sel(csz, E.cplane(1.0, w), cs, w=w)
    ct = E.tt(E.ts(tb, -1.0, OP.mult, w=w), css, OP.divide, w=w)
    ct2 = E.tt(ct, ct, OP.mult, w=w)
    ct2 = E.ts(ct2, 1.0, OP.add, w=w)
    sn1a = E.recip(E.sqrt(ct2, w=w), w=w)
    cs1a = E.tt(ct, sn1a, OP.mult, w=w)
    tbz = E.ts(tb, 0.0, OP.is_equal, w=w)
    tbs = E.sel(tbz, E.cplane(1.0, w), tb, w=w)
    tn = E.tt(E.ts(cs, -1.0, OP.mult, w=w), tbs, OP.divide, w=w)
    tn2 = E.tt(tn, tn, OP.mult, w=w)
    tn2 = E.ts(tn2, 1.0, OP.add, w=w)
    cs1b = E.recip(E.sqrt(tn2, w=w), w=w)
    sn1b = E.tt(tn, cs1b, OP.mult, w=w)
    agtm = E.tt(acs, ab, OP.is_gt, w=w)
    cs1 = E.sel(agtm, cs1a, E.sel(abz, E.cplane(1.0, w), cs1b, w=w), w=w)
    sn1 = E.sel(agtm, sn1a, E.sel(abz, E.cplane(0.0, w), sn1b, w=w), w=w)
    swap = E.tt(sgn1, sgn2, OP.is_equal, w=w)
    cs1f = E.sel(swap, E.ts(sn1, -1.0, OP.mult, w=w), cs1, w=w)
    sn1f = E.sel(swap, cs1, sn1, w=w)
    return rt1, rt2, cs1f, sn1f


def r3(zplane):
    """[128, 12] Z-col plane viewed as [128, 3, 4]."""
    return zplane[:].rearrange("p (r t) -> p r t", r=3)


def b3(cplane):
    """[128, 4] plane broadcast to [128, 3, 4] (step-0 middle dim)."""
    return cplane[:].unsqueeze(1).to_broadcast([128, 3, 4])


def build_program(for_sim=False):
    nc = bacc.Bacc("TRN2", target_bir_lowering=False) if not for_sim else bass.Bass()
    # ---------------- I/O ----------------
    lhsT5 = nc.dram_tensor("lhsT5", [5, NPC], FP, kind="ExternalInput")
    rhs5 = nc.dram_tensor("rhs5", [5, N], FP, kind="ExternalInput")
    ptsrec = nc.dram_tensor("ptsrec", [N * 64], FP, kind="ExternalInput")
    qx_in = nc.dram_tensor("qx", [128, NT], FP, kind="ExternalInput")
    qy_in = nc.dram_tensor("qy", [128, NT], FP, kind="ExternalInput")
    qz_in = nc.dram_tensor("qz", [128, NT], FP, kind="ExternalInput")
    qid_in = nc.dram_tensor("qid", [128, NT], FP, kind="ExternalInput")
    yg_in = nc.dram_tensor("ygrid", [128, W], FP, kind="ExternalInput")
    out = nc.dram_tensor("out", [NPC], FP, kind="ExternalOutput")
    dbg = nc.dram_tensor("dbg", [NPC, 8], FP, kind="ExternalOutput")

    idxscr = nc.dram_tensor("idxscr", [NT, 128 * KSL], I16)
    norm_local = nc.dram_tensor("norm_local", [NPC * 64], FP)
    norm_full = nc.dram_tensor("norm_full", [N * 64], FP, addr_space="Shared")
    norm_tab = nc.dram_tensor("norm_tab", [N * 64], FP)

    from contextlib import ExitStack
    with TileContext(nc) as tc, ExitStack() as _es:
        pool = _es.enter_context(tc.tile_pool(name="main", bufs=1))
        big = _es.enter_context(tc.tile_pool(name="big", bufs=1))
        psum = _es.enter_context(tc.tile_pool(name="psum", bufs=2, space="PSUM"))
        E = Emit(nc, tc, pool)
        KLVL = int(os.environ.get("KLVL", "9"))

        nc.gpsimd.load_library(library_config.mlp)
        # ---- load shared inputs ----
        lhsT_sb = pool.tile([5, NPC], FP, tag="lhsT", name="lhsT")
        rhs_sb = pool.tile([5, N], FP, tag="rhs", name="rhs")
        nc.sync.dma_start(out=lhsT_sb[:], in_=lhsT5[:])
        nc.sync.dma_start(out=rhs_sb[:], in_=rhs5[:])
        qx = pool.tile([128, NT], FP, tag="qx", name="qx"); nc.sync.dma_start(out=qx[:], in_=qx_in[:])
        qy = pool.tile([128, NT], FP, tag="qy", name="qy"); nc.sync.dma_start(out=qy[:], in_=qy_in[:])
        qz = pool.tile([128, NT], FP, tag="qz", name="qz"); nc.sync.dma_start(out=qz[:], in_=qz_in[:])
        qid = pool.tile([128, NT], FP, tag="qid", name="qid"); nc.sync.dma_start(out=qid[:], in_=qid_in[:])
        yg = pool.tile([128, W], FP, tag="yg", name="yg"); nc.sync.dma_start(out=yg[:], in_=yg_in[:])

        # persistent per-tile slot data
        nbx = [pool.tile([128, KSL], FP, tag=f"nbx{t}", name=f"nbx{t}") for t in range(NT)]
        nby = [pool.tile([128, KSL], FP, tag=f"nby{t}", name=f"nby{t}") for t in range(NT)]
        nbz = [pool.tile([128, KSL], FP, tag=f"nbz{t}", name=f"nbz{t}") for t in range(NT)]
        mask56 = [pool.tile([128, KSL], FP, tag=f"m56{t}", name=f"m56{t}") for t in range(NT)]
        gidxf56 = [pool.tile([128, KSL], FP, tag=f"g56{t}", name=f"g56{t}") for t in range(NT)]
        dtx = [pool.tile([128, KSL], FP, tag=f"dtx{t}", name=f"dtx{t}") for t in range(NT)]
        dty = [pool.tile([128, KSL], FP, tag=f"dty{t}", name=f"dty{t}") for t in range(NT)]

        # per-point planes
        def plane(tag):
            return pool.tile([128, NT], FP, tag=tag, name=tag)
        a00 = plane("a00"); a10 = plane("a10"); a11 = plane("a11")
        a20 = plane("a20"); a21 = plane("a21"); a22 = plane("a22")
        bbxmin = plane("bbxmin"); bbymin = plane("bbymin")
        maxlen = plane("maxlen")
        counts = plane("counts")
        xx00 = plane("xx00"); xx01 = plane("xx01"); xx11 = plane("xx11")
        yx00 = plane("yx00"); yx01 = plane("yx01"); yx10 = plane("yx10"); yx11 = plane("yx11")

        # ---------------- Phase 1+2+3: negD, selection, keys ----------------
        for t in range(NT):
            negd = big.tile([128, N], FP, tag="negd", name="negd")
            for c in range(NCH):
                ps = psum.tile([128, CH], FP, tag="ps", name="ps")
                nc.tensor.matmul(ps[:], lhsT=lhsT_sb[:, t * 128:(t + 1) * 128],
                                 rhs=rhs_sb[:, c * CH:(c + 1) * CH], start=True, stop=True)
                nc.scalar.copy(out=negd[:, c * CH:(c + 1) * CH], in_=ps[:])
            uval = big.tile([128, 128], FP, tag="uval", name="uval")
            ugid = big.tile([128, 128], FP, tag="ugid", name="ugid")
            scr = big.tile([128, CH], FP, tag="scr", name="scr")
            i8 = big.tile([128, 8], U32, tag="i8", name="i8")
            i8f = big.tile([128, 8], FP, tag="i8f", name="i8f")
            for c in range(NCH):
                chunk = negd[:, c * CH:(c + 1) * CH]
                base = c * TOPT
                # round 0
                nc.vector.max(out=uval[:, base:base + 8], in_=chunk)
                nc.vector.max_index(out=i8[:], in_max=uval[:, base:base + 8], in_values=chunk)
                nc.vector.tensor_copy(i8f[:], i8[:])
                nc.vector.tensor_scalar(out=ugid[:, base:base + 8], in0=i8f[:],
                                        scalar1=float(c * CH), scalar2=None, op0=OP.add)
                nc.vector.match_replace(out=scr[:], in_to_replace=uval[:, base:base + 8],
                                        in_values=chunk, imm_value=-3e38)
                # round 1
                nc.vector.max(out=uval[:, base + 8:base + 16], in_=scr[:])
                nc.vector.max_index(out=i8[:], in_max=uval[:, base + 8:base + 16], in_values=chunk)
                nc.vector.tensor_copy(i8f[:], i8[:])
                nc.vector.tensor_scalar(out=ugid[:, base + 8:base + 16], in0=i8f[:],
                                        scalar1=float(c * CH), scalar2=None, op0=OP.add)
            # theta = 50th largest of union
            work = big.tile([128, 128], FP, tag="work", name="work")
            nc.vector.tensor_copy(work[:], uval[:])
            s8 = big.tile([128, 64], FP, tag="s8", name="s8")
            for r in range(7):
                nc.vector.max(out=s8[:, r * 8:(r + 1) * 8], in_=work[:])
                if r < 6:
                    nc.vector.match_replace(out=work[:], in_to_replace=s8[:, r * 8:(r + 1) * 8],
                                            in_values=work[:], imm_value=-3e38)
            theta = s8[:, 49:50]
            # keys: sel*(2e6) + gidx - 1e6 ; extract top-56
            selm = big.tile([128, 128], FP, tag="selm", name="selm")
            nc.vector.tensor_scalar(out=selm[:], in0=uval[:], scalar1=theta, scalar2=None, op0=OP.is_ge)
            key = big.tile([128, 128], FP, tag="key", name="key")
            nc.vector.scalar_tensor_tensor(out=key[:], in0=selm[:], scalar=2.0e6, in1=ugid[:],
                                           op0=OP.mult, op1=OP.add)
            nc.vector.tensor_scalar(out=key[:], in0=key[:], scalar1=1.0e6, scalar2=None, op0=OP.subtract)
            k56 = big.tile([128, KSL], FP, tag="k56", name="k56")
            for r in range(7):
                nc.vector.max(out=k56[:, r * 8:(r + 1) * 8], in_=key[:])
                if r < 6:
                    nc.vector.match_replace(out=key[:], in_to_replace=k56[:, r * 8:(r + 1) * 8],
                                            in_values=key[:], imm_value=-3e38)
            nc.vector.tensor_scalar(out=mask56[t][:], in0=k56[:], scalar1=0.0, scalar2=None, op0=OP.is_ge)
            # gidx = key + 1e6 - 2e6*mask
            tmp = big.tile([128, KSL], FP, tag="tmpg", name="tmpg")
            nc.vector.scalar_tensor_tensor(out=tmp[:], in0=mask56[t][:], scalar=-2.0e6, in1=k56[:],
                                           op0=OP.mult, op1=OP.add)
            nc.vector.tensor_scalar(out=gidxf56[t][:], in0=tmp[:], scalar1=1.0e6, scalar2=None, op0=OP.add)
            nc.vector.tensor_scalar(out=gidxf56[t][:], in0=gidxf56[t][:], scalar1=0.0, scalar2=float(N - 1), op0=OP.max, op1=OP.min)
            # idx16 -> DRAM (flat transpose) -> wrapped [16, 448]
            idx16 = big.tile([128, KSL], I16, tag="idx16", name="idx16")
            nc.vector.tensor_copy(idx16[:], gidxf56[t][:])
            nc.sync.dma_start(out=idxscr[t].rearrange("(p s) -> p s", p=128), in_=idx16[:])

        if KLVL >= 2:
            # ---------------- Phase 4: gather neighbor coords ----------------
            for t in range(NT):
                idxw = big.tile([128, (128 * KSL) // 16], I16, tag="idxw", name="idxw")
                for g in range(8):
                    nc.sync.dma_start(out=idxw[g * 16:g * 16 + 16, :].rearrange("p (s q) -> p s q", q=8),
                                      in_=idxscr[t].rearrange("(q p s) -> p s q", p=16, s=KSL))
                rec = big.tile([128, KSL * 64], FP, tag="rec", name="rec")
                for sc in range(KSL // 8):
                                     nc.gpsimd.dma_gather(out_ap=rec[:].rearrange("p (s w) -> p s w", w=64)[:, sc * 8:(sc + 1) * 8, :],
                                                          in_ap=ptsrec[:].rearrange("(n w) -> n w", w=64),
                                                          idxs_ap=idxw[:, sc * 64:(sc + 1) * 64],
                                                          num_idxs=1024, num_idxs_reg=1024, elem_size=64, single_packet=False)
                rv = rec[:].rearrange("p (s w) -> p s w", s=KSL)
                nc.vector.tensor_copy(nbx[t][:], rv[:, :, 0:1])
                nc.vector.tensor_copy(nby[t][:], rv[:, :, 1:2])
                nc.vector.tensor_copy(nbz[t][:], rv[:, :, 2:3])

                # ---- cov (phase 5) ----
                m = mask56[t]
                mx = E.tt(nbx[t], m, OP.mult, w=KSL)
                my = E.tt(nby[t], m, OP.mult, w=KSL)
                mz = E.tt(nbz[t], m, OP.mult, w=KSL)
                sx = E.reduce(mx, OP.add); sy = E.reduce(my, OP.add); sz = E.reduce(mz, OP.add)
                mux = E.ts(sx, 1.0 / 50.0, OP.mult, w=1)
                muy = E.ts(sy, 1.0 / 50.0, OP.mult, w=1)
                muz = E.ts(sz, 1.0 / 50.0, OP.mult, w=1)
                cx_ = E.stt(nbx[t], mux[:, 0:1], m, OP.subtract, OP.mult, w=KSL)
                cy_ = E.stt(nby[t], muy[:, 0:1], m, OP.subtract, OP.mult, w=KSL)
                cz_ = E.stt(nbz[t], muz[:, 0:1], m, OP.subtract, OP.mult, w=KSL)
                for (p1, p2, dst) in ((cx_, cx_, a00), (cy_, cx_, a10), (cy_, cy_, a11),
                                      (cz_, cx_, a20), (cz_, cy_, a21), (cz_, cz_, a22)):
                    pr = E.tt(p1, p2, OP.mult, w=KSL)
                    s_ = E.reduce(pr, OP.add)
                    nc.vector.tensor_scalar(out=dst[:, t:t + 1], in0=s_[:], scalar1=0.5,
                                            scalar2=None, op0=OP.mult)

        if KLVL >= 3:
            # ---------------- Phase 6: eigh state machine ----------------
            w4 = NT
            alpha = a10
            xnorm = E.abs(a20)
            pyt = slapy2_em(E, alpha, xnorm)
            sgn_a = E.ts(alpha, 0.0, OP.is_ge, s2=2.0, op2=OP.mult)
            sgn_a = E.ts(sgn_a, -1.0, OP.add)
            beta = E.tt(E.ts(sgn_a, -1.0, OP.mult), pyt, OP.mult)
            hasref = E.ts(xnorm, 0.0, OP.not_equal)
            betasafe = E.sel(hasref, beta, E.cplane(1.0))
            tau0 = E.tt(E.tt(beta, alpha, OP.subtract), betasafe, OP.divide)
            tau0 = E.sel(hasref, tau0, E.cplane(0.0))
            tau = E.named(name="tauP")
            nc.vector.tensor_copy(tau[:], tau0[:])
            dab = E.tt(alpha, beta, OP.subtract)
            dabsafe = E.sel(hasref, dab, E.cplane(1.0))
            v1_0 = E.tt(a20, dabsafe, OP.divide)
            v1_0 = E.sel(hasref, v1_0, E.cplane(0.0))
            v1 = E.named(name="v1P")
            nc.vector.tensor_copy(v1[:], v1_0[:])
            e0 = E.sel(hasref, beta, a10)
            y1 = E.tt(tau, a11, OP.mult)
            y2 = E.tt(tau, a21, OP.mult)
            temp2 = E.tt(a21, v1, OP.mult)
            y1 = E.tt(y1, E.tt(tau, temp2, OP.mult), OP.add)
            y2 = E.tt(y2, E.tt(E.tt(tau, v1, OP.mult), a22, OP.mult), OP.add)
            sdot = E.tt(y1, E.tt(y2, v1, OP.mult), OP.add)
            alpha_c = E.tt(E.ts(tau, -0.5, OP.mult), sdot, OP.mult)
            y1 = E.tt(y1, alpha_c, OP.add)
            y2 = E.tt(y2, E.tt(alpha_c, v1, OP.mult), OP.add)
            ny1 = E.ts(y1, -1.0, OP.mult)
            a11n = E.tt(a11, E.tt(ny1, y1, OP.subtract), OP.add)
            a21n = E.tt(a21, E.tt(E.tt(v1, ny1, OP.mult), y2, OP.subtract), OP.add)
            ny2 = E.ts(y2, -1.0, OP.mult)
            a22n = E.tt(a22, E.tt(E.tt(v1, ny2, OP.mult), E.tt(y2, v1, OP.mult), OP.subtract), OP.add)
            d0 = E.copy(a00)
            d1 = E.sel(hasref, a11n, a11)
            d2 = E.sel(hasref, a22n, a22)
            e1 = E.sel(hasref, a21n, a21)
            # direction / reversal
            rev0 = E.tt(E.abs(d2), E.abs(d0), OP.is_lt)
            rev = E.named(name="revP")
            nc.vector.tensor_copy(rev[:], rev0[:])
            d0r = E.sel(rev, d2, d0); d2r = E.sel(rev, d0, d2)
            e0r = E.sel(rev, e1, e0); e1r = E.sel(rev, e0, e1)
            d0, d2, e0, e1 = d0r, d2r, e0r, e1r
            rev12 = E.named(12, name="rev12P")
            nc.vector.tensor_copy(r3(rev12), b3(rev))
            # Z columns
            zc = []
            for colinit in range(3):
                z = pool.tile([128, 12], FP, tag=f"zc{colinit}", name=f"zc{colinit}")
                nc.vector.memset(z[:], 0.0)
                nc.vector.memset(z[:, colinit * 4:(colinit + 1) * 4], 1.0)
                zc.append(z)
            active = E.cplane(1.0)
            active = E.copy(active)
            b01 = E.copy(E.cplane(0.0)); b12 = E.copy(E.cplane(0.0))

            for it in range(6):
                ad0 = E.abs(d0); ad1 = E.abs(d1); ad2 = E.abs(d2)
                e0sq = E.tt(e0, e0, OP.mult)
                thr0 = E.ts(ad0, float(EPS2), OP.mult)
                thr0 = E.tt(thr0, ad1, OP.mult)
                thr0 = E.ts(thr0, float(SAFMIN), OP.add)
                t_e0 = E.tt(e0sq, thr0, OP.is_le)
                e1sq = E.tt(e1, e1, OP.mult)
                thr1 = E.ts(ad1, float(EPS2), OP.mult)
                thr1 = E.tt(thr1, ad2, OP.mult)
                thr1 = E.ts(thr1, float(SAFMIN), OP.add)
                t_e1 = E.tt(e1sq, thr1, OP.is_le)
                not0 = E.ts(t_e0, 0.0, OP.is_equal)
                not1 = E.ts(t_e1, 0.0, OP.is_equal)
                fire1 = E.tt(active, t_e0, OP.mult)
                fire2 = E.tt(E.tt(active, not0, OP.mult), t_e1, OP.mult)
                sweepm = E.tt(E.tt(active, not0, OP.mult), not1, OP.mult)
                b12 = E.tt(b12, E.tt(fire1, not1, OP.mult), OP.add)
                b01 = E.tt(b01, fire2, OP.add)
                active = sweepm
                m = sweepm
                esafe0 = E.sel(m, e0, E.cplane(1.0))
                g = E.tt(E.tt(d1, d0, OP.subtract), E.ts(esafe0, 2.0, OP.mult), OP.divide)
                r = slapy2_em(E, g, E.cplane(1.0))
                gge = E.ts(g, 0.0, OP.is_ge)
                srg = E.sel(gge, r, E.ts(r, -1.0, OP.mult))
                den2 = E.tt(g, srg, OP.add)
                g = E.tt(E.tt(d2, d0, OP.subtract), E.tt(esafe0, den2, OP.divide), OP.add)
                # i = 2
                Fv = E.copy(e1); Bv = E.copy(e1)
                c_, s_, rr_ = slartg_em(E, g, Fv)
                gq = E.copy(d2)               # d2 - p with p=0
                t1_ = E.tt(d1, gq, OP.subtract)
                rq = E.tt(E.tt(t1_, s_, OP.mult), E.tt(E.ts(c_, 2.0, OP.mult), Bv, OP.mult), OP.add)
                p_ = E.tt(s_, rq, OP.mult)
                d2 = E.sel(m, E.tt(gq, p_, OP.add), d2)
                g = E.tt(E.tt(c_, rq, OP.mult), Bv, OP.subtract)
                c1s = c_; s1s = s_
                # i = 1
                Fv = E.tt(s_, e0, OP.mult); Bv = E.tt(c_, e0, OP.mult)
                c_, s_, rr_ = slartg_em(E, g, Fv)
                e1 = E.sel(m, rr_, e1)
                gq = E.tt(d1, p_, OP.subtract)
                t1_ = E.tt(d0, gq, OP.subtract)
                rq = E.tt(E.tt(t1_, s_, OP.mult), E.tt(E.ts(c_, 2.0, OP.mult), Bv, OP.mult), OP.add)
                p2_ = E.tt(s_, rq, OP.mult)
                d1 = E.sel(m, E.tt(gq, p2_, OP.add), d1)
                g = E.tt(E.tt(c_, rq, OP.mult), Bv, OP.subtract)
                d0 = E.sel(m, E.tt(d0, p2_, OP.subtract), d0)
                e0 = E.sel(m, g, e0)
                # Z rotations: cur (1,2) with (c1,s1); cur (0,1) with (c2,s2)
                for (cc, ss, which) in ((c1s, s1s, "12"), (c_, s_, "01")):
                    ccn = E.sel(m, cc, E.cplane(1.0))
                    ssn = E.sel(m, ss, E.cplane(0.0))
                    ccb = E.fresh(12); nc.vector.tensor_copy(r3(ccb), b3(ccn))
                    ssb = E.fresh(12); nc.vector.tensor_copy(r3(ssb), b3(ssn))
                    X = zc[1]
                    if which == "12":
                        Y = E.sel(rev12, zc[0], zc[2], w=12)
                        Ynew = E.tt(E.tt(ccb, Y, OP.mult, w=12), E.tt(ssb, X, OP.mult, w=12), OP.add, w=12)
                        Xnew = E.tt(E.tt(ccb, X, OP.mult, w=12), E.tt(ssb, Y, OP.mult, w=12), OP.subtract, w=12)
                    else:
                        Y = E.sel(rev12, zc[2], zc[0], w=12)
                        Xnew = E.tt(E.tt(ccb, X, OP.mult, w=12), E.tt(ssb, Y, OP.mult, w=12), OP.add, w=12)
                        Ynew = E.tt(E.tt(ccb, Y, OP.mult, w=12), E.tt(ssb, X, OP.mult, w=12), OP.subtract, w=12)
                    nc.vector.tensor_copy(zc[1][:], Xnew[:])
                    if which == "12":
                        z0n = E.sel(rev12, Ynew, zc[0], w=12)
                        z2n = E.sel(rev12, zc[2], Ynew, w=12)
                    else:
                        z0n = E.sel(rev12, zc[0], Ynew, w=12)
                        z2n = E.sel(rev12, Ynew, zc[2], w=12)
                    nc.vector.tensor_copy(zc[0][:], z0n[:])
                    nc.vector.tensor_copy(zc[2][:], z2n[:])

            # terminal
            d0f = E.sel(rev, d2, d0); d2f = E.sel(rev, d0, d2)
            e0f = E.sel(rev, e1, e0); e1f = E.sel(rev, e0, e1)
            b01f = E.sel(rev, b12, b01); b12f = E.sel(rev, b01, b12)
            d0, d2 = d0f, d2f
            av = E.sel(b12f, d1, d0); bv = E.sel(b12f, e1f, e0f); cv = E.sel(b12f, d2, d1)
            rt1, rt2, cs1, sn1 = slaev2_em(E, av, bv, cv)
            hasrot = E.tt(b01f, b12f, OP.add)
            csn = E.sel(hasrot, cs1, E.cplane(1.0))
            snn = E.sel(hasrot, sn1, E.cplane(0.0))
            csb = E.fresh(12); nc.vector.tensor_copy(r3(csb), b3(csn))
            snb = E.fresh(12); nc.vector.tensor_copy(r3(snb), b3(snn))
            b12b = E.fresh(12); nc.vector.tensor_copy(r3(b12b), b3(b12f))
            b01b = E.fresh(12); nc.vector.tensor_copy(r3(b01b), b3(b01f))
            HIGH = E.sel(b12b, zc[2], zc[1], w=12)
            LOW = E.sel(b12b, zc[1], zc[0], w=12)
            Hn = E.tt(E.tt(csb, HIGH, OP.mult, w=12), E.tt(snb, LOW, OP.mult, w=12), OP.subtract, w=12)
            Ln = E.tt(E.tt(snb, HIGH, OP.mult, w=12), E.tt(csb, LOW, OP.mult, w=12), OP.add, w=12)
            z0n = E.sel(b01b, Ln, zc[0], w=12)
            z1n = E.sel(b01b, Hn, E.sel(b12b, Ln, zc[1], w=12), w=12)
            z2n = E.sel(b12b, Hn, zc[2], w=12)
            nc.vector.tensor_copy(zc[0][:], z0n[:])
            nc.vector.tensor_copy(zc[1][:], z1n[:])
            nc.vector.tensor_copy(zc[2][:], z2n[:])
            d0 = E.sel(b01f, rt1, d0)
            d1 = E.sel(b01f, rt2, E.sel(b12f, rt1, d1))
            d2 = E.sel(b12f, rt2, d2)
            # selection sort (ascending, strict <)
            c1_ = E.tt(d1, d0, OP.is_lt)
            cand = E.sel(c1_, d1, d0)
            c2_ = E.tt(d2, cand, OP.is_lt)
            k_is1 = E.tt(c1_, E.ts(c2_, 0.0, OP.is_equal), OP.mult)
            k_is2 = c2_
            # swap 0<->k
            d0n = E.sel(k_is2, d2, E.sel(k_is1, d1, d0))
            d1n = E.sel(k_is1, d0, d1)
            d2n = E.sel(k_is2, d0, d2)
            d0, d1, d2 = d0n, d1n, d2n
            k1b = E.fresh(12); nc.vector.tensor_copy(r3(k1b), b3(k_is1))
            k2b = E.fresh(12); nc.vector.tensor_copy(r3(k2b), b3(k_is2))
            z0n = E.sel(k2b, zc[2], E.sel(k1b, zc[1], zc[0], w=12), w=12)
            z1n = E.sel(k1b, zc[0], zc[1], w=12)
            z2n = E.sel(k2b, zc[0], zc[2], w=12)
            nc.vector.tensor_copy(zc[0][:], z0n[:])
            nc.vector.tensor_copy(zc[1][:], z1n[:])
            nc.vector.tensor_copy(zc[2][:], z2n[:])
            # pass 2: swap 1<->2 if d2 < d1
            mm = E.tt(d2, d1, OP.is_lt)
            d1n = E.sel(mm, d2, d1); d2n = E.sel(mm, d1, d2)
            d1, d2 = d1n, d2n
            mmb = E.fresh(12); nc.vector.tensor_copy(r3(mmb), b3(mm))
            z1n = E.sel(mmb, zc[2], zc[1], w=12)
            z2n = E.sel(mmb, zc[1], zc[2], w=12)
            nc.vector.tensor_copy(zc[1][:], z1n[:])
            nc.vector.tensor_copy(zc[2][:], z2n[:])
            # sormtr: apply H to rows 1,2 of each column
            tv = E.tt(tau, v1, OP.mult)
            for col in range(3):
                z = zc[col]
                r1v = z[:, 4:8]; r2v = z[:, 8:12]
                wv_ = E.tt(r1v, E.tt(v1, r2v, OP.mult), OP.add)
                nc.vector.tensor_tensor(out=z[:, 4:8], in0=r1v, in1=E.tt(tau, wv_, OP.mult)[:], op=OP.subtract)
                nc.vector.tensor_tensor(out=z[:, 8:12], in0=r2v, in1=E.tt(tv, wv_, OP.mult)[:], op=OP.subtract)
            # frames: row0 = col0 (normal), row1 = col1 * det, row2 = col2
            nx_, ny_, nz_ = zc[0][:, 0:4], zc[0][:, 4:8], zc[0][:, 8:12]
            u1x, u1y, u1z = zc[1][:, 0:4], zc[1][:, 4:8], zc[1][:, 8:12]
            u2x, u2y, u2z = zc[2][:, 0:4], zc[2][:, 4:8], zc[2][:, 8:12]
            crx = E.tt(E.tt(u1y, u2z, OP.mult), E.tt(u1z, u2y, OP.mult), OP.subtract)
            cry = E.tt(E.tt(u1z, u2x, OP.mult), E.tt(u1x, u2z, OP.mult), OP.subtract)
            crz = E.tt(E.tt(u1x, u2y, OP.mult), E.tt(u1y, u2x, OP.mult), OP.subtract)
            det = E.tt(E.tt(nx_, crx, OP.mult), E.tt(E.tt(ny_, cry, OP.mult), E.tt(nz_, crz, OP.mult), OP.add), OP.add)
            dsgn = E.ts(det, 0.0, OP.is_ge, s2=2.0, op2=OP.mult)
            dsgn = E.ts(dsgn, -1.0, OP.add)
            t1x = E.named(name="t1xP"); t1y = E.named(name="t1yP"); t1z = E.named(name="t1zP")
            E.tt(u1x, dsgn, OP.mult, out=t1x); E.tt(u1y, dsgn, OP.mult, out=t1y); E.tt(u1z, dsgn, OP.mult, out=t1z)

        if KLVL >= 4:
            # ---------------- Phase 7: normals exchange ----------------
            for (k, pl) in ((0, (nx_, None)), (1, (ny_, None)), (2, (nz_, None))):
                src = pl[0]
                dst = bass.AP(norm_local.tensor if hasattr(norm_local, "tensor") else norm_local[:].tensor,
                              k, [[64, 128], [8192, NT]]) if False else None
            # simpler: rearranged AP on the dram tensor
            zpad = pool.tile([128, NT * 64], FP, tag="zpad", name="zpad")
            nc.vector.memset(zpad[:], 0.0)
            nl = norm_local[:].rearrange("(q w) -> q w", w=64)
            nc.sync.dma_start(out=nl.rearrange("(t p) w -> p t w", p=128), in_=zpad[:].rearrange("p (t w) -> p t w", w=64))
            nc.sync.dma_start(out=nl[:, 0:1].rearrange("(t p) w -> p t w", p=128), in_=nx_.unsqueeze(2))
            nc.sync.dma_start(out=nl[:, 1:2].rearrange("(t p) w -> p t w", p=128), in_=ny_.unsqueeze(2))
            nc.sync.dma_start(out=nl[:, 2:3].rearrange("(t p) w -> p t w", p=128), in_=nz_.unsqueeze(2))
            nc.gpsimd.collective_compute(
                "AllGather", OP.bypass, replica_groups=[list(range(NCORE))],
                ins=[norm_local[:]], outs=[norm_full[:]],
            )
            nfstage = pool.tile([128, N * 64 // 128], FP, tag="nfstage", name="nfstage")
            nc.sync.dma_start(out=nfstage[:], in_=norm_full[:].rearrange("(p f) -> p f", p=128))
            nc.sync.dma_start(out=norm_tab[:].rearrange("(p f) -> p f", p=128), in_=nfstage[:])

        if KLVL >= 5:
            # ---------------- Phase 8+9+10 per tile ----------------
            gauss_pl = plane("gauss"); area_pl = plane("area")
            for t in range(NT):
                m = mask56[t]
                dfx = E.ts(nbx[t], qx[:, t:t + 1], OP.subtract, w=KSL)
                dfy = E.ts(nby[t], qy[:, t:t + 1], OP.subtract, w=KSL)
                dfz = E.ts(nbz[t], qz[:, t:t + 1], OP.subtract, w=KSL)
                dtx_t = E.ts(dfx, t1x[:, t:t + 1], OP.mult, w=KSL)
                dtx_t = E.stt(dfy, t1y[:, t:t + 1], dtx_t, OP.mult, OP.add, w=KSL)
                dtx_t = E.stt(dfz, t1z[:, t:t + 1], dtx_t, OP.mult, OP.add, w=KSL)
                nc.vector.tensor_copy(dtx[t][:], dtx_t[:])
                dty_t = E.ts(dfx, u2x[:, t:t + 1], OP.mult, w=KSL)
                dty_t = E.stt(dfy, u2y[:, t:t + 1], dty_t, OP.mult, OP.add, w=KSL)
                dty_t = E.stt(dfz, u2z[:, t:t + 1], dty_t, OP.mult, OP.add, w=KSL)
                nc.vector.tensor_copy(dty[t][:], dty_t[:])
                # bbox
                big_p = E.cplane(1e30, KSL); big_n = E.cplane(-1e30, KSL)
                vx = E.sel(m, dtx[t], big_p, w=KSL)
                E.reduce(vx, OP.min, out=bbxmin[:, t:t + 1])
                vy = E.sel(m, dty[t], big_p, w=KSL)
                E.reduce(vy, OP.min, out=bbymin[:, t:t + 1])
                vx = E.sel(m, dtx[t], big_n, w=KSL)
                bxmax = E.reduce(vx, OP.max)
                vy = E.sel(m, dty[t], big_n, w=KSL)
                bymax = E.reduce(vy, OP.max)
                nc.vector.tensor_scalar(out=bbxmin[:, t:t + 1], in0=bbxmin[:, t:t + 1], scalar1=1.1, scalar2=None, op0=OP.mult)
                nc.vector.tensor_scalar(out=bbymin[:, t:t + 1], in0=bbymin[:, t:t + 1], scalar1=1.1, scalar2=None, op0=OP.mult)
                bxmax = E.ts(bxmax, 1.1, OP.mult, w=1)
                bymax = E.ts(bymax, 1.1, OP.mult, w=1)
                rx = E.tt(bxmax, bbxmin[:, t:t + 1], OP.subtract, w=1)
                ry = E.tt(bymax, bbymin[:, t:t + 1], OP.subtract, w=1)
                E.tt(rx, ry, OP.max, out=maxlen[:, t:t + 1], w=1)

            for t in range(NT):
                m = mask56[t]
                # coords
                cx_ = E.ts(dtx[t], bbxmin[:, t:t + 1], OP.subtract, w=KSL)
                cx_ = E.ts(cx_, maxlen[:, t:t + 1], OP.divide, w=KSL)
                cx_ = E.ts(cx_, 2.0, OP.mult, s2=-1.0, op2=OP.add, w=KSL)
                cy_ = E.ts(dty[t], bbymin[:, t:t + 1], OP.subtract, w=KSL)
                cy_ = E.ts(cy_, maxlen[:, t:t + 1], OP.divide, w=KSL)
                cy_ = E.ts(cy_, 2.0, OP.mult, s2=-1.0, op2=OP.add, w=KSL)
                c0x = E.ts(bbxmin[:, t:t + 1], -1.0, OP.mult, w=1)
                c0x = E.ts(c0x, maxlen[:, t:t + 1], OP.divide, w=1)
                c0x = E.ts(c0x, 2.0, OP.mult, s2=-1.0, op2=OP.add, w=1)
                c0y = E.ts(bbymin[:, t:t + 1], -1.0, OP.mult, w=1)
                c0y = E.ts(c0y, maxlen[:, t:t + 1], OP.divide, w=1)
                c0y = E.ts(c0y, 2.0, OP.mult, s2=-1.0, op2=OP.add, w=1)
